# revision 2
# baseline (speedup 1.0000x reference)
"""Trainium2 Bass kernel for nn_Aligner (head-summed sparse attention), v2.

Math (per batch b):
  Q = hs @ Wq + bq            [LQ, 384]
  K = x  @ Wk + bk            [LK, 384]
  V = x  @ Wv + bv            [LK, 384]
  S = Q @ K^T / sqrt(192)     (head-sum of per-head scores == full-width dot)
  P = softmax_k(S + (m-1)*inf)
  out = P @ V                 [LQ, 384]

Restructurings (all exact up to fp):
- No-max softmax: scores are O(1), P = exp(S*scale)*m normalized by the
  row sum at the end (ones column of the PV rhs).
- S^T [k, q] layout: probability tiles come out pre-transposed for the
  P^T-stationary PX matmul.
- K-associativity: S^T = X @ (Wk @ Q'^T). R = Wk@Q'^T is a tiny [384, LQ]
  matmul, eliminating the K projection (bk drops: softmax-invariant).
- V-associativity: out = (P@[X|1]) @ Wv + bv. The row sum falls out of
  the ones column; the V projection collapses into one final
  [LQ,384]@[384,384] matmul.
- Multiplicative masking: P = exp(S*scale) .* m post-exp on DVE (exact:
  m in {0,1}), replacing the additive -1e9 trick entirely.
- The S^T contraction runs 2/3 in fp8e4m3 DoubleRow (x^T features 0:256
  paired with R rows 0:256 at 2 MACs/cycle) and 1/3 in bf16.

Host-side data prep (pure layout/dtype, no math): all dense inputs are
pre-cast to bf16; x^T arrives pre-transposed with features 0:256 packed
into fp8e4 DoubleRow pairs and 256:384 as bf16; the mask arrives
pre-transposed [k, q] as bf16 {0,1}. This removes every device-side
transpose and cast from the main loop and cuts HBM traffic ~2.6x.

Schedule: per 512-wide k segment, loads for segment N+1 are issued while
segment N's 4 chunks run score->exp->mask->PV; PV matmuls trail their
scores by 2 chunks so the exp+mask latency hides behind the next chunk's
score matmuls; a ~3us junk-matmul warm-up during the initial DMA wait
keeps the HAM clock gate open. DMAs spread across SP/ACT/DVE queues.

Sharding: 8 cores = batch(4) x LQ-halves(2); no collectives.
"""

import math
from contextlib import ExitStack

import numpy as np

import concourse.bass as bass
import concourse.tile as tile
from concourse import bacc, mybir
from concourse.bass_utils import run_bass_kernel_spmd
from concourse.masks import make_identity

B, LQ_FULL, LK, HID = 4, 1024, 4096, 384
LQ = LQ_FULL // 2    # per-core q shard
P = 128
NHC = HID // P       # 3 feature chunks
SEG = 512            # k segment width
NSEG = LK // SEG     # 8
NKC = LK // P        # 32 k chunks
NQS = LQ // P        # 4 q subtiles
NJ = SEG // P        # 4 chunks per segment
SCALE = 1.0 / math.sqrt(192.0)

F32 = mybir.dt.float32
F8E4 = mybir.dt.float8e4
F8E5 = mybir.dt.float8e5
BF16 = mybir.dt.bfloat16

_CACHE = {}


def _body(tc, ctx, d, pfx=""):
    nc = tc.nc
    AF = mybir.ActivationFunctionType

    consts = ctx.enter_context(tc.tile_pool(name=f"consts{pfx}", bufs=1))
    stage = ctx.enter_context(tc.tile_pool(name=f"stage{pfx}", bufs=3))
    outp = ctx.enter_context(tc.tile_pool(name=f"outp{pfx}", bufs=2))
    mmps = ctx.enter_context(tc.tile_pool(name=f"mmps{pfx}", bufs=3, space="PSUM"))
    pvps = ctx.enter_context(tc.tile_pool(name=f"pvps{pfx}", bufs=1, space="PSUM"))

    # PE warm-up: junk matmuls during the initial DMA wait keep the HAM
    # clock gate open so the prologue runs at full clock.
    wtiny = consts.tile([P, 256], BF16, name="wtiny", tag="wtiny")
    nc.vector.memset(wtiny, 0.0)
    wu = mmps.tile([P, 256], F32, name="wu", tag="mm")
    for i in range(9):
        nc.tensor.matmul(wu, lhsT=wtiny[:, 0:P], rhs=wtiny,
                         start=True, stop=True)

    # touch Exp once so ACT's table load lands in the idle head
    warm = consts.tile([P, 1], F32, name="warm", tag="warm")
    nc.vector.memset(warm, 0.0)
    warm2 = consts.tile([P, 1], F32, name="warm2", tag="warm2")
    nc.scalar.activation(out=warm2, in_=warm, func=AF.Exp, scale=1.0)

    # ---- prologue loads (bf16, host-precast): the R chain gates the
    # first score, so the weight loads go first on both queues ----
    wqtt = consts.tile([P, NHC, HID], BF16, name="wqtt", tag="wqtt")
    nc.sync.dma_start(out=wqtt, in_=d["wqt"].rearrange("(c p) h -> p c h", p=P))
    wktt = consts.tile([P, NHC, HID], BF16, name="wktt", tag="wktt")
    nc.scalar.dma_start(out=wktt, in_=d["wkt"].rearrange("(c p) h -> p c h", p=P))
    bq_sb = consts.tile([P, NHC], F32, name="bq_sb", tag="bq_sb")
    nc.sync.dma_start(out=bq_sb, in_=d["bq"].rearrange("(c p) -> p c", p=P))
    hstt = consts.tile([P, NHC, LQ], BF16, name="hstt", tag="hstt")
    nc.scalar.dma_start(out=hstt, in_=d["hst"].rearrange("(c p) q -> p c q", p=P))

    # ---- per-superseg (1024 k) loads: x rows (+ones col), mask^T fp8,
    # x^T fp8 pairs, x^T bf16 tail. One DMA per tensor per superseg keeps
    # the HWDGE descriptor-generation overhead (~630ns/DMA) off the
    # critical path. Superseg 0 splits into 512-halves, score-side
    # tensors first, so the first chunks unblock during the Q/R chain.
    SS = 2 * SEG         # 1024
    NSJ = SS // P        # 8 chunks per superseg
    NSS = LK // SS       # 4

    def load_ss(ss):
        k0 = ss * SS
        fine = ss == 0
        xseg = stage.tile([P, NSJ, HID + 1], BF16, name="xseg", tag="xseg", bufs=2)
        xsrc = d["xb"][k0:k0 + SS, :].rearrange("(j p) h -> p j h", p=P)
        mtseg = stage.tile([P, NSJ, LQ], F8E5, name="mtseg", tag="mtseg", bufs=2)
        msrc = d["mt"][k0:k0 + SS, :].rearrange("(j p) q -> p j q", p=P)
        xtps = stage.tile([P, 2, SS], F8E4, name="xtps", tag="xtps", bufs=2)
        xt2s = stage.tile([P, SS], BF16, name="xt2s", tag="xt2s", bufs=2)
        if fine:
            # everything on the sync queue: the scalar queue must stay
            # clear for the Q/R chain's ACT copies (DMA dispatch occupies
            # the issuing engine's SEQ for ~667ns each)
            for j0 in range(0, NSJ, 4):
                nc.sync.dma_start(out=xtps[:, :, j0 * P:(j0 + 4) * P],
                                  in_=d["xtp"][:, :, k0 + j0 * P:k0 + (j0 + 4) * P])
                nc.sync.dma_start(out=xt2s[:, j0 * P:(j0 + 4) * P],
                                  in_=d["xt2"][:, k0 + j0 * P:k0 + (j0 + 4) * P])
                nc.sync.dma_start(out=mtseg[:, j0:j0 + 4, :], in_=msrc[:, j0:j0 + 4, :])
            for j0 in range(0, NSJ, 4):
                nc.sync.dma_start(out=xseg[:, j0:j0 + 4, 0:HID], in_=xsrc[:, j0:j0 + 4, :])
        else:
            nc.sync.dma_start(out=xtps, in_=d["xtp"][:, :, k0:k0 + SS])
            nc.scalar.dma_start(out=xt2s, in_=d["xt2"][:, k0:k0 + SS])
            nc.scalar.dma_start(out=mtseg, in_=msrc)
            nc.sync.dma_start(out=xseg[:, :, 0:HID], in_=xsrc)
        nc.vector.memset(xseg[:, :, HID:HID + 1], 1.0)
        return dict(xseg=xseg, mtseg=mtseg, xtps=xtps, xt2s=xt2s, k0=k0)

    s0 = load_ss(0)

    # identity for the epilogue transposes (DVE memset: first GPSIMD op has
    # ~1.4us launch latency and nothing PE-side waits on gpsimd here)
    ident = consts.tile([P, P], BF16, name="ident", tag="ident")
    nc.vector.memset(ident, 0.0)
    make_identity(nc, ident, nomemset=True)

    # ---- W2^T = Wq @ Wk^T [h, h']: weight-only, starts before hs/x land.
    # Folding Wq into the R matmul shortens the fill-critical chain:
    # R = W2^T.T @ hs^T + Wk@bq. ----
    w2t = []
    for hc in range(NHC):
        ps_w = mmps.tile([P, HID], F32, name="ps_w", tag="mm")
        for fc in range(NHC):
            nc.tensor.matmul(
                ps_w, lhsT=wqtt[:, fc, hc * P:(hc + 1) * P], rhs=wktt[:, fc, :],
                start=(fc == 0), stop=(fc == NHC - 1),
            )
        t = consts.tile([P, HID], BF16, name=f"w2t{hc}", tag=f"w2t{hc}")
        if hc == 1:
            nc.vector.tensor_copy(out=t, in_=ps_w)
        else:
            nc.scalar.activation(out=t, in_=ps_w, func=AF.Copy)
        w2t.append(t)

    # c = Wk @ bq as per-partition columns [P, NHC] (R's bias port input)
    bqb = consts.tile([P, NHC], BF16, name="bqb", tag="bqb")
    nc.vector.tensor_copy(out=bqb, in_=bq_sb)
    c_sb = consts.tile([P, NHC], F32, name="c_sb", tag="c_sb")
    for hc in range(NHC):
        ps_c = mmps.tile([P, 1], F32, name="ps_c", tag="mm")
        for fc in range(NHC):
            nc.tensor.matmul(
                ps_c, lhsT=wktt[:, fc, hc * P:(hc + 1) * P], rhs=bqb[:, fc:fc + 1],
                start=(fc == 0), stop=(fc == NHC - 1),
            )
        nc.vector.tensor_copy(out=c_sb[:, hc:hc + 1], in_=ps_c)

    # ---- R = W2^T.T @ hs^T + c  [h', q]: rows 0:256 as fp8e4 DoubleRow
    # pairs, rows 256:384 bf16 (hybrid keeps softmax noise in budget) ----
    rp0 = consts.tile([P, 2, LQ], F8E4, name="rp0", tag="rp0")
    r2 = consts.tile([P, LQ], BF16, name="r2", tag="r2")
    for hc in range(NHC):
        ps_r = mmps.tile([P, LQ], F32, name="ps_r", tag="mm")
        for fc in range(NHC):
            nc.tensor.matmul(
                ps_r, lhsT=w2t[fc][:, hc * P:(hc + 1) * P], rhs=hstt[:, fc, :],
                start=(fc == 0), stop=(fc == NHC - 1),
            )
        if hc < 2:
            nc.scalar.activation(out=rp0[:, hc, :], in_=ps_r, func=AF.Identity,
                                 bias=c_sb[:, hc:hc + 1], scale=1.0)
        else:
            nc.scalar.activation(out=r2, in_=ps_r, func=AF.Identity,
                                 bias=c_sb[:, hc:hc + 1], scale=1.0)

    pv_ps = [pvps.tile([P, HID + 1], F32, name=f"pv{qs}", tag=f"pv{qs}")
             for qs in range(NQS)]

    # ---- per-chunk: scores (PE), exp (ACT), mask mult (DVE), PV (PE,
    # issued 2 chunks later so the exp+mask latency hides) ----
    def score(sg, j):
        kc = sg["k0"] // P + j
        st = mmps.tile([P, LQ], F32, name="st", tag="mm")
        nc.tensor.matmul(
            st, lhsT=sg["xtps"][:, :, j * P:(j + 1) * P], rhs=rp0,
            start=True, stop=False,
            perf_mode=mybir.MatmulPerfMode.DoubleRow,
        )
        nc.tensor.matmul(
            st, lhsT=sg["xt2s"][:, j * P:(j + 1) * P], rhs=r2,
            start=False, stop=True,
        )
        pt = stage.tile([P, LQ], BF16, name="pt", tag="pt", bufs=3)
        nc.scalar.activation(out=pt, in_=st, func=AF.Exp, scale=SCALE)
        ptm = stage.tile([P, LQ], BF16, name="ptm", tag="ptm", bufs=4)
        nc.vector.tensor_mul(out=ptm, in0=pt, in1=sg["mtseg"][:, j, :])

        def pv():
            for qs in range(NQS):
                nc.tensor.matmul(
                    pv_ps[qs], lhsT=ptm[:, qs * P:(qs + 1) * P],
                    rhs=sg["xseg"][:, j, :],
                    start=(kc == 0), stop=(kc == NKC - 1),
                )
        return pv

    PV_DEPTH = 2
    pvq = []

    def push_pv(pv):
        pvq.append(pv)
        if len(pvq) > PV_DEPTH:
            pvq.pop(0)()

    prev = s0
    for ss in range(1, NSS):
        cur = load_ss(ss)
        for j in range(NSJ):
            push_pv(score(prev, j))
        prev = cur
    for j in range(NSJ):
        push_pv(score(prev, j))
    for pv in pvq:
        pv()

    # ---- epilogue: Wv/bv (loads overlap the tail of the main loop),
    # normalize via ACT scale-port (recips early on DVE), transpose,
    # Wv projection + bv via a static ones-row matmul, DMA from PSUM.
    wvt = consts.tile([P, NHC, HID], BF16, name="wvt", tag="wvt")
    nc.sync.dma_start(out=wvt, in_=d["wv"].rearrange("(c p) h -> p c h", p=P))
    bv_d = d["bvr"]
    bvr = outp.tile([1, HID], BF16, name="bvr", tag="bvr", bufs=1)
    nc.sync.dma_start(
        out=bvr,
        in_=bass.AP(tensor=bv_d.tensor, offset=bv_d.offset, ap=[[0, 1], [1, HID]]),
    )
    onesr = outp.tile([1, P], BF16, name="onesr", tag="onesr", bufs=1)
    nc.vector.memset(onesr, 1.0)

    recips = []
    for qs in range(NQS):
        r = stage.tile([P, 1], F32, name="r", tag="r", bufs=4)
        nc.vector.reciprocal(out=r, in_=pv_ps[qs][:, HID:HID + 1])
        recips.append(r)
    pxn = []
    for qs in range(NQS):
        t = outp.tile([P, HID + 1], BF16, name=f"pxn{qs}", tag=f"pxn{qs}", bufs=1)
        if qs % 2 == 0:
            nc.scalar.activation(out=t, in_=pv_ps[qs], func=AF.Copy, scale=recips[qs])
        else:
            nc.vector.tensor_scalar_mul(out=t, in0=pv_ps[qs], scalar1=recips[qs])
        pxn.append(t)
    # qs-major transposes: qs=0's column can flow while pxn[1..3] copy
    tps = [mmps.tile([P, LQ], BF16, name=f"tp_p{hc}", tag="mm") for hc in range(NHC)]
    for qs in range(NQS):
        for hc in range(NHC):
            nc.tensor.transpose(tps[hc][:, qs * P:(qs + 1) * P],
                                pxn[qs][:, hc * P:(hc + 1) * P], ident)
    pxnt = []
    for hc in range(NHC):
        t = outp.tile([P, LQ], BF16, name=f"pxnt{hc}", tag=f"pxnt{hc}", bufs=1)
        if hc == 1:
            nc.vector.tensor_copy(out=t, in_=tps[hc])
        else:
            nc.scalar.activation(out=t, in_=tps[hc], func=AF.Copy)
        pxnt.append(t)
    # hc-major projection: po[qs] accumulate in the freed PV banks; the
    # bv row lands first, then each feature chunk as its pxnt arrives.
    po = [pvps.tile([P, HID], F32, name=f"po{qs}", tag=f"pv{qs}")
          for qs in range(NQS)]
    for qs in range(NQS):
        nc.tensor.matmul(po[qs], lhsT=onesr, rhs=bvr, start=True, stop=False)
    for hc in range(NHC):
        for qs in range(NQS):
            nc.tensor.matmul(
                po[qs], lhsT=pxnt[hc][:, qs * P:(qs + 1) * P], rhs=wvt[:, hc, :],
                start=False, stop=(hc == NHC - 1),
            )
    o = outp.tile([P, NQS, HID], BF16, name="o", tag="o", bufs=1)
    odst = d["out"].rearrange("(qs p) h -> p qs h", p=P)
    for qs in range(NQS):
        if qs % 2 == 0:
            nc.vector.tensor_copy(out=o[:, qs, :], in_=po[qs])
        else:
            nc.scalar.activation(out=o[:, qs, :], in_=po[qs], func=AF.Copy)
        if qs == 1:
            nc.sync.dma_start(out=odst[:, 0:2, :], in_=o[:, 0:2, :])
    nc.sync.dma_start(out=odst[:, 2:4, :], in_=o[:, 2:4, :])


def _build(repeats=1):
    if ("nc", repeats) in _CACHE:
        return _CACHE["nc", repeats]
    nc = bacc.Bacc(
        "TRN2", target_bir_lowering=False, debug=False,
        enable_asserts=False, num_devices=8,
    )
    d = {
        "hst": nc.dram_tensor("hst", [HID, LQ], BF16, kind="ExternalInput").ap(),
        "wqt": nc.dram_tensor("wqt", [HID, HID], BF16, kind="ExternalInput").ap(),
        "wkt": nc.dram_tensor("wkt", [HID, HID], BF16, kind="ExternalInput").ap(),
        "bq": nc.dram_tensor("bq", [HID], F32, kind="ExternalInput").ap(),
        "wv": nc.dram_tensor("wv", [HID, HID], BF16, kind="ExternalInput").ap(),
        "bvr": nc.dram_tensor("bvr", [HID], BF16, kind="ExternalInput").ap(),
        "xb": nc.dram_tensor("xb", [LK, HID], BF16, kind="ExternalInput").ap(),
        "xtp": nc.dram_tensor("xtp", [P, 2, LK], F8E4, kind="ExternalInput").ap(),
        "xt2": nc.dram_tensor("xt2", [P, LK], BF16, kind="ExternalInput").ap(),
        "mt": nc.dram_tensor("mt", [LK, LQ], F8E5, kind="ExternalInput").ap(),
        "out": nc.dram_tensor("out", [LQ, HID], BF16, kind="ExternalOutput").ap(),
    }
    with tile.TileContext(nc) as tc:
        for rep in range(repeats):
            with ExitStack() as ctx:
                _body(tc, ctx, d, pfx=f"_{rep}" if repeats > 1 else "")
    nc.compile()
    _CACHE["nc", repeats] = nc
    return nc


LAST_RESULTS = None


def _in_maps(hidden_states, right_hidden_states, attention_mask,
             Wq, bq, Wk, bk, Wv, bv):
    import ml_dtypes
    BF = ml_dtypes.bfloat16
    F8 = ml_dtypes.float8_e4m3
    F8M = ml_dtypes.float8_e5m2

    hs_all = np.asarray(hidden_states, np.float32)
    x_all = np.asarray(right_hidden_states, np.float32)
    m_all = np.asarray(attention_mask)
    wqt_b = np.ascontiguousarray(np.asarray(Wq, np.float32).T).astype(BF)
    wkt_b = np.ascontiguousarray(np.asarray(Wk, np.float32).T).astype(BF)
    wv_b = np.asarray(Wv, np.float32).astype(BF)
    bq_f = np.asarray(bq, np.float32)
    bv_b = np.asarray(bv, np.float32).astype(BF)

    per_batch = []
    for b in range(B):
        xb_b = x_all[b].astype(BF)                       # [LK, HID]
        xbT = np.ascontiguousarray(xb_b.T)               # [HID, LK] bf16
        xtp_b = np.ascontiguousarray(
            xbT[0:2 * P].reshape(2, P, LK).transpose(1, 0, 2)).astype(F8)
        xt2_b = np.ascontiguousarray(xbT[2 * P:3 * P])   # [P, LK] bf16
        per_batch.append((xb_b, xtp_b, xt2_b))

    in_maps = []
    for c in range(8):
        b, h = divmod(c, 2)
        sl = slice(h * LQ, (h + 1) * LQ)
        xb_b, xtp_b, xt2_b = per_batch[b]
        in_maps.append({
            "hst": np.ascontiguousarray(hs_all[b, sl].T).astype(BF),
            "wqt": wqt_b, "wkt": wkt_b, "bq": bq_f,
            "wv": wv_b, "bvr": bv_b,
            "xb": xb_b, "xtp": xtp_b, "xt2": xt2_b,
            "mt": np.ascontiguousarray(m_all[b, sl].T).astype(F8M),
        })
    return in_maps


def kernel(hidden_states, right_hidden_states, attention_mask,
           Wq, bq, Wk, bk, Wv, bv):
    global LAST_RESULTS
    import os
    os.environ.setdefault("BASS_NEVER_TRACE", "1")
    nc = _build()
    in_maps = _in_maps(hidden_states, right_hidden_states, attention_mask,
                       Wq, bq, Wk, bk, Wv, bv)
    res = run_bass_kernel_spmd(nc, in_maps, core_ids=list(range(8)))
    LAST_RESULTS = res
    out = np.empty((B, LQ_FULL, HID), np.float32)
    for c in range(8):
        b, h = divmod(c, 2)
        out[b, h * LQ:(h + 1) * LQ] = res.results[c]["out"].astype(np.float32)
    return out


# revision 3
# speedup vs baseline: 11.2797x; 11.2797x over previous
"""Trainium2 Bass kernel for nn_Aligner (head-summed sparse attention), v2.

Math (per batch b):
  Q = hs @ Wq + bq            [LQ, 384]
  K = x  @ Wk + bk            [LK, 384]
  V = x  @ Wv + bv            [LK, 384]
  S = Q @ K^T / sqrt(192)     (head-sum of per-head scores == full-width dot)
  P = softmax_k(S + (m-1)*inf)
  out = P @ V                 [LQ, 384]

Restructurings (all exact up to fp):
- No-max softmax: scores are O(1), P = exp(S*scale)*m normalized by the
  row sum at the end (ones column of the PV rhs).
- S^T [k, q] layout: probability tiles come out pre-transposed for the
  P^T-stationary PX matmul.
- K-associativity: S^T = X @ (Wk @ Q'^T). R = Wk@Q'^T is a tiny [384, LQ]
  matmul, eliminating the K projection (bk drops: softmax-invariant).
- V-associativity: out = (P@[X|1]) @ Wv + bv. The row sum falls out of
  the ones column; the V projection collapses into one final
  [LQ,384]@[384,384] matmul.
- Multiplicative masking: P = exp(S*scale) .* m post-exp on DVE (exact:
  m in {0,1}), replacing the additive -1e9 trick entirely.
- The S^T contraction runs 2/3 in fp8e4m3 DoubleRow (x^T features 0:256
  paired with R rows 0:256 at 2 MACs/cycle) and 1/3 in bf16.

Host-side data prep (pure layout/dtype, no math): all dense inputs are
pre-cast to bf16; x^T arrives pre-transposed with features 0:256 packed
into fp8e4 DoubleRow pairs and 256:384 as bf16; the mask arrives
pre-transposed [k, q] as bf16 {0,1}. This removes every device-side
transpose and cast from the main loop and cuts HBM traffic ~2.6x.

Schedule: per 512-wide k segment, loads for segment N+1 are issued while
segment N's 4 chunks run score->exp->mask->PV; PV matmuls trail their
scores by 2 chunks so the exp+mask latency hides behind the next chunk's
score matmuls; a ~3us junk-matmul warm-up during the initial DMA wait
keeps the HAM clock gate open. DMAs spread across SP/ACT/DVE queues.

Sharding: 8 cores = batch(4) x LQ-halves(2); no collectives.
"""

import math
from contextlib import ExitStack

import numpy as np

import concourse.bass as bass
import concourse.tile as tile
from concourse import bacc, mybir
from concourse.bass_utils import run_bass_kernel_spmd
from concourse.masks import make_identity

B, LQ_FULL, LK, HID = 4, 1024, 4096, 384
LQ = LQ_FULL // 2    # per-core q shard
P = 128
NHC = HID // P       # 3 feature chunks
SEG = 512            # k segment width
NSEG = LK // SEG     # 8
NKC = LK // P        # 32 k chunks
NQS = LQ // P        # 4 q subtiles
NJ = SEG // P        # 4 chunks per segment
SCALE = 1.0 / math.sqrt(192.0)

F32 = mybir.dt.float32
F8E4 = mybir.dt.float8e4
F8E5 = mybir.dt.float8e5
BF16 = mybir.dt.bfloat16

_CACHE = {}


def _body(tc, ctx, d, pfx=""):
    nc = tc.nc
    AF = mybir.ActivationFunctionType

    consts = ctx.enter_context(tc.tile_pool(name=f"consts{pfx}", bufs=1))
    stage = ctx.enter_context(tc.tile_pool(name=f"stage{pfx}", bufs=3))
    outp = ctx.enter_context(tc.tile_pool(name=f"outp{pfx}", bufs=2))
    mmps = ctx.enter_context(tc.tile_pool(name=f"mmps{pfx}", bufs=3, space="PSUM"))
    pvps = ctx.enter_context(tc.tile_pool(name=f"pvps{pfx}", bufs=1, space="PSUM"))

    # PE warm-up: junk matmuls during the initial DMA wait keep the HAM
    # clock gate open so the prologue runs at full clock.
    wtiny = consts.tile([P, 256], BF16, name="wtiny", tag="wtiny")
    nc.vector.memset(wtiny, 0.0)
    wu = mmps.tile([P, 256], F32, name="wu", tag="mm")
    for i in range(9):
        nc.tensor.matmul(wu, lhsT=wtiny[:, 0:P], rhs=wtiny,
                         start=True, stop=True)

    # touch Exp once so ACT's table load lands in the idle head
    warm = consts.tile([P, 1], F32, name="warm", tag="warm")
    nc.vector.memset(warm, 0.0)
    warm2 = consts.tile([P, 1], F32, name="warm2", tag="warm2")
    nc.scalar.activation(out=warm2, in_=warm, func=AF.Exp, scale=1.0)

    # ---- prologue loads (bf16, host-precast): the R chain gates the
    # first score, so the weight loads go first on both queues ----
    wqtt = consts.tile([P, NHC, HID], BF16, name="wqtt", tag="wqtt")
    nc.sync.dma_start(out=wqtt, in_=d["wqt"].rearrange("(c p) h -> p c h", p=P))
    wktt = consts.tile([P, NHC, HID], BF16, name="wktt", tag="wktt")
    nc.scalar.dma_start(out=wktt, in_=d["wkt"].rearrange("(c p) h -> p c h", p=P))
    bq_sb = consts.tile([P, NHC], F32, name="bq_sb", tag="bq_sb")
    nc.sync.dma_start(out=bq_sb, in_=d["bq"].rearrange("(c p) -> p c", p=P))
    hstt = consts.tile([P, NHC, LQ], BF16, name="hstt", tag="hstt")
    nc.scalar.dma_start(out=hstt, in_=d["hst"].rearrange("(c p) q -> p c q", p=P))

    # ---- per-superseg (1024 k) loads: x rows (+ones col), mask^T fp8,
    # x^T fp8 pairs, x^T bf16 tail. One DMA per tensor per superseg keeps
    # the HWDGE descriptor-generation overhead (~630ns/DMA) off the
    # critical path. Superseg 0 splits into 512-halves, score-side
    # tensors first, so the first chunks unblock during the Q/R chain.
    SS = 2 * SEG         # 1024
    NSJ = SS // P        # 8 chunks per superseg
    NSS = LK // SS       # 4

    def load_ss(ss):
        k0 = ss * SS
        fine = ss == 0
        xseg = stage.tile([P, NSJ, HID + 1], BF16, name="xseg", tag="xseg", bufs=2)
        xsrc = d["xb"][k0:k0 + SS, :].rearrange("(j p) h -> p j h", p=P)
        mtseg = stage.tile([P, NSJ, LQ], F8E5, name="mtseg", tag="mtseg", bufs=2)
        msrc = d["mt"][k0:k0 + SS, :].rearrange("(j p) q -> p j q", p=P)
        xtps = stage.tile([P, 2, SS], F8E4, name="xtps", tag="xtps", bufs=2)
        xt2s = stage.tile([P, 2, SS], F8E4, name="xt2s", tag="xt2s", bufs=2)
        if fine:
            # everything on the sync queue: the scalar queue must stay
            # clear for the Q/R chain's ACT copies (DMA dispatch occupies
            # the issuing engine's SEQ for ~667ns each)
            for j0 in range(0, NSJ, 4):
                nc.sync.dma_start(out=xtps[:, :, j0 * P:(j0 + 4) * P],
                                  in_=d["xtp"][:, :, k0 + j0 * P:k0 + (j0 + 4) * P])
                nc.sync.dma_start(out=xt2s[:, :, j0 * P:(j0 + 4) * P],
                                  in_=d["xt2"][:, :, k0 + j0 * P:k0 + (j0 + 4) * P])
                nc.sync.dma_start(out=mtseg[:, j0:j0 + 4, :], in_=msrc[:, j0:j0 + 4, :])
            for j0 in range(0, NSJ, 4):
                nc.sync.dma_start(out=xseg[:, j0:j0 + 4, 0:HID], in_=xsrc[:, j0:j0 + 4, :])
        else:
            nc.sync.dma_start(out=xtps, in_=d["xtp"][:, :, k0:k0 + SS])
            nc.scalar.dma_start(out=xt2s, in_=d["xt2"][:, :, k0:k0 + SS])
            nc.scalar.dma_start(out=mtseg, in_=msrc)
            nc.sync.dma_start(out=xseg[:, :, 0:HID], in_=xsrc)
        nc.vector.memset(xseg[:, :, HID:HID + 1], 1.0)
        return dict(xseg=xseg, mtseg=mtseg, xtps=xtps, xt2s=xt2s, k0=k0)

    s0 = load_ss(0)

    # identity for the epilogue transposes (DVE memset: first GPSIMD op has
    # ~1.4us launch latency and nothing PE-side waits on gpsimd here)
    ident = consts.tile([P, P], BF16, name="ident", tag="ident")
    nc.vector.memset(ident, 0.0)
    make_identity(nc, ident, nomemset=True)

    # ---- W2^T = Wq @ Wk^T [h, h']: weight-only, starts before hs/x land.
    # Folding Wq into the R matmul shortens the fill-critical chain:
    # R = W2^T.T @ hs^T + Wk@bq. ----
    w2t = []
    for hc in range(NHC):
        ps_w = mmps.tile([P, HID], F32, name="ps_w", tag="mm")
        for fc in range(NHC):
            nc.tensor.matmul(
                ps_w, lhsT=wqtt[:, fc, hc * P:(hc + 1) * P], rhs=wktt[:, fc, :],
                start=(fc == 0), stop=(fc == NHC - 1),
            )
        t = consts.tile([P, HID], BF16, name=f"w2t{hc}", tag=f"w2t{hc}")
        if hc == 1:
            nc.vector.tensor_copy(out=t, in_=ps_w)
        else:
            nc.scalar.activation(out=t, in_=ps_w, func=AF.Copy)
        w2t.append(t)

    # c = Wk @ bq as per-partition columns [P, NHC] (R's bias port input)
    bqb = consts.tile([P, NHC], BF16, name="bqb", tag="bqb")
    nc.vector.tensor_copy(out=bqb, in_=bq_sb)
    c_sb = consts.tile([P, NHC], F32, name="c_sb", tag="c_sb")
    for hc in range(NHC):
        ps_c = mmps.tile([P, 1], F32, name="ps_c", tag="mm")
        for fc in range(NHC):
            nc.tensor.matmul(
                ps_c, lhsT=wktt[:, fc, hc * P:(hc + 1) * P], rhs=bqb[:, fc:fc + 1],
                start=(fc == 0), stop=(fc == NHC - 1),
            )
        nc.vector.tensor_copy(out=c_sb[:, hc:hc + 1], in_=ps_c)

    # ---- R = W2^T.T @ hs^T + c  [h', q]: rows 0:256 as fp8e4 DoubleRow
    # pairs, rows 256:384 bf16 (hybrid keeps softmax noise in budget) ----
    rp0 = consts.tile([P, 2, LQ], F8E4, name="rp0", tag="rp0")
    rp2 = consts.tile([P, 2, LQ], F8E4, name="rp2", tag="rp2")
    nc.vector.memset(rp2[:, 1, :], 0.0)
    for hc in range(NHC):
        ps_r = mmps.tile([P, LQ], F32, name="ps_r", tag="mm")
        for fc in range(NHC):
            nc.tensor.matmul(
                ps_r, lhsT=w2t[fc][:, hc * P:(hc + 1) * P], rhs=hstt[:, fc, :],
                start=(fc == 0), stop=(fc == NHC - 1),
            )
        if hc < 2:
            nc.scalar.activation(out=rp0[:, hc, :], in_=ps_r, func=AF.Identity,
                                 bias=c_sb[:, hc:hc + 1], scale=1.0)
        else:
            nc.scalar.activation(out=rp2[:, 0, :], in_=ps_r, func=AF.Identity,
                                 bias=c_sb[:, hc:hc + 1], scale=1.0)

    pv_ps = [pvps.tile([P, HID + 1], F32, name=f"pv{qs}", tag=f"pv{qs}")
             for qs in range(NQS)]

    # ---- per-chunk: scores (PE), exp (ACT), mask mult (DVE), PV (PE,
    # issued 2 chunks later so the exp+mask latency hides) ----
    def score(sg, j):
        kc = sg["k0"] // P + j
        st = mmps.tile([P, LQ], F32, name="st", tag="mm")
        nc.tensor.matmul(
            st, lhsT=sg["xtps"][:, :, j * P:(j + 1) * P], rhs=rp0,
            start=True, stop=False,
            perf_mode=mybir.MatmulPerfMode.DoubleRow,
        )
        nc.tensor.matmul(
            st, lhsT=sg["xt2s"][:, :, j * P:(j + 1) * P], rhs=rp2,
            start=False, stop=True,
            perf_mode=mybir.MatmulPerfMode.DoubleRow,
        )
        pt = stage.tile([P, LQ], BF16, name="pt", tag="pt", bufs=3)
        nc.scalar.activation(out=pt, in_=st, func=AF.Exp, scale=SCALE)
        ptm = stage.tile([P, LQ], BF16, name="ptm", tag="ptm", bufs=4)
        nc.vector.tensor_mul(out=ptm, in0=pt, in1=sg["mtseg"][:, j, :])

        def pv():
            for qs in range(NQS):
                nc.tensor.matmul(
                    pv_ps[qs], lhsT=ptm[:, qs * P:(qs + 1) * P],
                    rhs=sg["xseg"][:, j, :],
                    start=(kc == 0), stop=(kc == NKC - 1),
                )
        return pv

    PV_DEPTH = 2
    pvq = []

    def push_pv(pv):
        pvq.append(pv)
        if len(pvq) > PV_DEPTH:
            pvq.pop(0)()

    prev = s0
    for ss in range(1, NSS):
        cur = load_ss(ss)
        for j in range(NSJ):
            push_pv(score(prev, j))
        prev = cur
    for j in range(NSJ):
        push_pv(score(prev, j))
    for pv in pvq:
        pv()

    # ---- epilogue: Wv/bv (loads overlap the tail of the main loop),
    # normalize via ACT scale-port (recips early on DVE), transpose,
    # Wv projection + bv via a static ones-row matmul, DMA from PSUM.
    wvt = consts.tile([P, NHC, HID], BF16, name="wvt", tag="wvt")
    nc.sync.dma_start(out=wvt, in_=d["wv"].rearrange("(c p) h -> p c h", p=P))
    bv_d = d["bvr"]
    bvr = outp.tile([1, HID], BF16, name="bvr", tag="bvr", bufs=1)
    nc.sync.dma_start(
        out=bvr,
        in_=bass.AP(tensor=bv_d.tensor, offset=bv_d.offset, ap=[[0, 1], [1, HID]]),
    )
    onesr = outp.tile([1, P], BF16, name="onesr", tag="onesr", bufs=1)
    nc.vector.memset(onesr, 1.0)

    recips = []
    for qs in range(NQS):
        r = stage.tile([P, 1], F32, name="r", tag="r", bufs=4)
        nc.vector.reciprocal(out=r, in_=pv_ps[qs][:, HID:HID + 1])
        recips.append(r)
    pxn = []
    for qs in range(NQS):
        t = outp.tile([P, HID + 1], BF16, name=f"pxn{qs}", tag=f"pxn{qs}", bufs=1)
        if qs % 2 == 0:
            nc.scalar.activation(out=t, in_=pv_ps[qs], func=AF.Copy, scale=recips[qs])
        else:
            nc.vector.tensor_scalar_mul(out=t, in0=pv_ps[qs], scalar1=recips[qs])
        pxn.append(t)
    # qs-major transposes: qs=0's column can flow while pxn[1..3] copy
    tps = [mmps.tile([P, LQ], BF16, name=f"tp_p{hc}", tag="mm") for hc in range(NHC)]
    for qs in range(NQS):
        for hc in range(NHC):
            nc.tensor.transpose(tps[hc][:, qs * P:(qs + 1) * P],
                                pxn[qs][:, hc * P:(hc + 1) * P], ident)
    # half-split copies across ACT+DVE so po's qs 0-1 matmuls can start
    # before the back half of each pxnt lands
    pxnt = []
    for hc in range(NHC):
        t = outp.tile([P, LQ], BF16, name=f"pxnt{hc}", tag=f"pxnt{hc}", bufs=1)
        nc.scalar.activation(out=t[:, 0:LQ // 2], in_=tps[hc][:, 0:LQ // 2],
                             func=AF.Copy)
        nc.vector.tensor_copy(out=t[:, LQ // 2:LQ], in_=tps[hc][:, LQ // 2:LQ])
        pxnt.append(t)
    # hc-major projection: po[qs] accumulate in the freed PV banks; the
    # bv row lands first, then each feature chunk as its pxnt arrives.
    po = [pvps.tile([P, HID], F32, name=f"po{qs}", tag=f"pv{qs}")
          for qs in range(NQS)]
    for qs in range(NQS):
        nc.tensor.matmul(po[qs], lhsT=onesr, rhs=bvr, start=True, stop=False)
    for hc in range(NHC):
        for qs in range(NQS):
            nc.tensor.matmul(
                po[qs], lhsT=pxnt[hc][:, qs * P:(qs + 1) * P], rhs=wvt[:, hc, :],
                start=False, stop=(hc == NHC - 1),
            )
    o = outp.tile([P, NQS, HID], BF16, name="o", tag="o", bufs=1)
    odst = d["out"].rearrange("(qs p) h -> p qs h", p=P)
    for qs in range(NQS):
        if qs % 2 == 0:
            nc.scalar.activation(out=o[:, qs, :], in_=po[qs], func=AF.Copy)
        else:
            nc.vector.tensor_copy(out=o[:, qs, :], in_=po[qs])
        if qs == 1:
            nc.sync.dma_start(out=odst[:, 0:2, :], in_=o[:, 0:2, :])
    nc.sync.dma_start(out=odst[:, 2:4, :], in_=o[:, 2:4, :])


def _build(repeats=1):
    if ("nc", repeats) in _CACHE:
        return _CACHE["nc", repeats]
    nc = bacc.Bacc(
        "TRN2", target_bir_lowering=False, debug=False,
        enable_asserts=False, num_devices=8,
    )
    d = {
        "hst": nc.dram_tensor("hst", [HID, LQ], BF16, kind="ExternalInput").ap(),
        "wqt": nc.dram_tensor("wqt", [HID, HID], BF16, kind="ExternalInput").ap(),
        "wkt": nc.dram_tensor("wkt", [HID, HID], BF16, kind="ExternalInput").ap(),
        "bq": nc.dram_tensor("bq", [HID], F32, kind="ExternalInput").ap(),
        "wv": nc.dram_tensor("wv", [HID, HID], BF16, kind="ExternalInput").ap(),
        "bvr": nc.dram_tensor("bvr", [HID], BF16, kind="ExternalInput").ap(),
        "xb": nc.dram_tensor("xb", [LK, HID], BF16, kind="ExternalInput").ap(),
        "xtp": nc.dram_tensor("xtp", [P, 2, LK], F8E4, kind="ExternalInput").ap(),
        "xt2": nc.dram_tensor("xt2", [P, 2, LK], F8E4, kind="ExternalInput").ap(),
        "mt": nc.dram_tensor("mt", [LK, LQ], F8E5, kind="ExternalInput").ap(),
        "out": nc.dram_tensor("out", [LQ, HID], BF16, kind="ExternalOutput").ap(),
    }
    with tile.TileContext(nc) as tc:
        for rep in range(repeats):
            with ExitStack() as ctx:
                _body(tc, ctx, d, pfx=f"_{rep}" if repeats > 1 else "")
    nc.compile()
    _CACHE["nc", repeats] = nc
    return nc


LAST_RESULTS = None


def _in_maps(hidden_states, right_hidden_states, attention_mask,
             Wq, bq, Wk, bk, Wv, bv):
    import ml_dtypes
    BF = ml_dtypes.bfloat16
    F8 = ml_dtypes.float8_e4m3
    F8M = ml_dtypes.float8_e5m2

    hs_all = np.asarray(hidden_states, np.float32)
    x_all = np.asarray(right_hidden_states, np.float32)
    m_all = np.asarray(attention_mask)
    wqt_b = np.ascontiguousarray(np.asarray(Wq, np.float32).T).astype(BF)
    wkt_b = np.ascontiguousarray(np.asarray(Wk, np.float32).T).astype(BF)
    wv_b = np.asarray(Wv, np.float32).astype(BF)
    bq_f = np.asarray(bq, np.float32)
    bv_b = np.asarray(bv, np.float32).astype(BF)

    per_batch = []
    for b in range(B):
        xb_b = x_all[b].astype(BF)                       # [LK, HID]
        xbT = np.ascontiguousarray(xb_b.T)               # [HID, LK] bf16
        xtp_b = np.ascontiguousarray(
            xbT[0:2 * P].reshape(2, P, LK).transpose(1, 0, 2)).astype(F8)
        xt2_b = np.zeros((P, 2, LK), dtype=F8)
        xt2_b[:, 0, :] = xbT[2 * P:3 * P].astype(F8)
        per_batch.append((xb_b, xtp_b, xt2_b))

    in_maps = []
    for c in range(8):
        b, h = divmod(c, 2)
        sl = slice(h * LQ, (h + 1) * LQ)
        xb_b, xtp_b, xt2_b = per_batch[b]
        in_maps.append({
            "hst": np.ascontiguousarray(hs_all[b, sl].T).astype(BF),
            "wqt": wqt_b, "wkt": wkt_b, "bq": bq_f,
            "wv": wv_b, "bvr": bv_b,
            "xb": xb_b, "xtp": xtp_b, "xt2": xt2_b,
            "mt": np.ascontiguousarray(m_all[b, sl].T).astype(F8M),
        })
    return in_maps


def kernel(hidden_states, right_hidden_states, attention_mask,
           Wq, bq, Wk, bk, Wv, bv):
    global LAST_RESULTS
    import os
    os.environ.setdefault("BASS_NEVER_TRACE", "1")
    nc = _build()
    in_maps = _in_maps(hidden_states, right_hidden_states, attention_mask,
                       Wq, bq, Wk, bk, Wv, bv)
    res = run_bass_kernel_spmd(nc, in_maps, core_ids=list(range(8)))
    LAST_RESULTS = res
    out = np.empty((B, LQ_FULL, HID), np.float32)
    for c in range(8):
        b, h = divmod(c, 2)
        out[b, h * LQ:(h + 1) * LQ] = res.results[c]["out"].astype(np.float32)
    return out
